# revision 10
# baseline (speedup 1.0000x reference)
"""Trainium2 Bass kernel for nn_LocalFeatureExtractor (gnn_message_passing).

Math: with per-node features x[b,n,:] (C=128) and K=10 gathered neighbors,
    out = x @ W1^T + W1_b + (conv(feats) + Wc_b) @ W2^T + W2_b
collapses algebraically (fold the two dense layers around the conv) to
    out[b,n] = x[b,n] @ A + sum_k x[b, adj[b,n,k]] @ M_k + bias
with A = W1^T + (W2 Wc_0)^T, M_k = (W2 Wc_k)^T, bias = W1_b + W2_b + W2 Wc_b.

Sharding: data-parallel over batch B=8 -> one graph per NeuronCore.

v3: the neighbor gather runs as NON-transpose SWDGE dma_gather (HBM source,
node-major landing) spread over all 4 SWDGE queues. Probed facts driving
this design:
  - transpose-mode gathers CANNOT run on >1 queue concurrently: the rx
    descriptors spray through the shared XBAR transpose unit and two
    queues' streams interleave its state (probe: 72-81% corruption, even
    with no ring reclaim in flight). Non-transpose gathers have no xbar
    involvement and probe clean on 2 and 4 queues under reclaim pressure.
  - each queue is served by its own GPSIMD Q7 core pair (ucode:
    cpu_id/2 == queue_num) and its own descriptor ring per SDMA engine, so
    4 queues ~ 4.8x one queue (probe: 6.6 -> 1.37 ns/idx single-core).
  - queue q must own a disjoint set of Tile DMASW completion sems, or ring
    reclaim on one queue can free slots whose transfer another queue still
    has in flight. Tile assigns DMASW sems round-robin over 8 lanes in
    program order of Pool-engine DMA instructions, so issuing gathers with
    queue = counter % 4 gives queue q sems {q, q+4}: disjoint.

Gathered data lands [node-on-partition, channel]; a PE transpose per
128-node chunk (matmul against a bf16 identity) re-orients it to
[channel, node] in PSUM, ACT/DVE copy-casts to bf16 SBUF, and the folded
[C,C] weight matmuls accumulate k=0..10 into per-strip PSUM chains.
"""

import numpy as np
import ml_dtypes

import concourse.bass as bass
import concourse.mybir as mybir
from concourse import bacc
from concourse.tile import TileContext
from concourse.bass_utils import run_bass_kernel_spmd

B, N, C, K = 8, 20000, 128, 10
N_CORES = 8
NQ = 4                      # SWDGE queues (gathers round-robin per call)
NB = 2560                   # nodes per gather block (40 calls/graph: 8|40
                            # so queue balance repeats exactly per rep)
STRIP = 512                 # PSUM accum strip = one 2KB fp32 bank
NPAD2 = 20096               # xT zero-padded so padded block cols hit zeros

_dt = mybir.dt
BF16 = ml_dtypes.bfloat16


def _blocks(n=N):
    """(node_offset, real_nodes, padded_nodes) per gather block."""
    out = []
    off = 0
    while off < n:
        nb = min(NB, n - off)
        out.append((off, nb, -(-nb // 128) * 128))
        off += nb
    return out


def build(n_cores=N_CORES, reps=1, n=N, gbufs=8, blk_limit=None, mode="full",
          nq=NQ):
    """Build + compile the per-core Bass program (SPMD: same program, 8 cores).

    mode: "full" | "gather_only" (ablation: skip transpose/copy/matmul k>0)
    """
    blocks = _blocks(n)
    if blk_limit is not None:
        blocks = blocks[:blk_limit]
    tot_slots = sum((K // 2) * (2 * nbp) // 16 for _, _, nbp in blocks)

    nc = bacc.Bacc("TRN2", target_bir_lowering=False, debug=False,
                   num_devices=n_cores, num_swdge_queues=NQ)
    x_rows = nc.dram_tensor("x_rows", [n, C], _dt.bfloat16, kind="ExternalInput").ap()
    xT = nc.dram_tensor("xT", [C, NPAD2], _dt.bfloat16, kind="ExternalInput").ap()
    idx = nc.dram_tensor("idx", [C, tot_slots], _dt.int16, kind="ExternalInput").ap()
    wts = nc.dram_tensor("wts", [C, (K + 1) * C], _dt.bfloat16, kind="ExternalInput").ap()
    bias = nc.dram_tensor("bias", [C, 1], _dt.float32, kind="ExternalInput").ap()
    ident = nc.dram_tensor("ident", [C, C], _dt.bfloat16, kind="ExternalInput").ap()
    outT = nc.dram_tensor("outT", [C, n], _dt.float16, kind="ExternalOutput").ap()

    RMAX = 2 * NB // 128    # gather-buf ranks (full block)

    with TileContext(nc) as tc:
        with tc.tile_pool(name="const", bufs=1) as cpool, \
             tc.tile_pool(name="gath", bufs=gbufs) as gpool, \
             tc.tile_pool(name="stage", bufs=3, space="PSUM") as spool, \
             tc.tile_pool(name="acc", bufs=1, space="PSUM") as apool, \
             tc.tile_pool(name="rhs", bufs=4) as rpool, \
             tc.tile_pool(name="outp", bufs=3) as opool:
            slots0 = (K // 2) * (2 * blocks[0][2]) // 16
            xT_t = cpool.tile([C, NPAD2], _dt.bfloat16)
            idxa_t = cpool.tile([C, slots0], _dt.int16)
            idxb_t = cpool.tile([C, tot_slots - slots0], _dt.int16)
            wts_t = cpool.tile([C, (K + 1) * C], _dt.bfloat16)
            bias_t = cpool.tile([C, 1], _dt.float32)
            id_t = cpool.tile([C, C], _dt.bfloat16)
            nc.sync.dma_start(out=idxa_t[:], in_=idx[:, 0:slots0])
            nc.sync.dma_start(out=idxb_t[:], in_=idx[:, slots0:])
            nc.sync.dma_start(out=wts_t[:], in_=wts[:])
            nc.sync.dma_start(out=id_t[:], in_=ident[:])
            nc.sync.dma_start(out=bias_t[:], in_=bias[:])
            nc.sync.dma_start(out=xT_t[:], in_=xT[:])

            def idx_slice(s0, s1):
                if s1 <= slots0:
                    return idxa_t[:, s0:s1]
                return idxb_t[:, s0 - slots0:s1 - slots0]

            # Queue choice must be a pure function of (program-order index
            # mod 8) so each queue owns a disjoint set of the 8 round-robin
            # DMASW sems. Any residue->queue map qualifies; this one pairs
            # heavy residues (r0/r1 carry 7 calls incl. a remainder call)
            # with light ones to balance per-queue gathered-index load
            # (50432 x3 / 49664 for NB=2560's 35x5120 + 5x4352 calls).
            qmap = [0, 1, 2, 0, 1, 2, 3, 3]
            qctr = 0     # global gather counter: queue qmap[qctr%8], sem qctr%8
            cctr = 0     # copy counter: alternate ACT / DVE
            for _rep in range(reps):
                scol = 0
                for (off, nb, nbp) in blocks:
                    gpc = 2 * nbp
                    ranks = nbp // 128          # ranks per k-slot
                    nstrip = -(-nbp // STRIP)
                    gs = []
                    for c in range(K // 2):
                        g = gpool.tile([C, RMAX, C], _dt.bfloat16, tag="g")
                        nc.gpsimd.dma_gather(
                            g[:, 0:2 * ranks, :], x_rows[:],
                            idx_slice(scol, scol + gpc // 16),
                            gpc, gpc, C, transpose=False,
                            single_packet=False,
                            queue_num=(qmap[qctr % 8] if nq == NQ else qctr % nq),
                        )
                        gs.append(g)
                        qctr += 1
                        scol += gpc // 16
                    pss = []
                    for j in range(nstrip):
                        cw = min(STRIP, nbp - j * STRIP)
                        ps = apool.tile([C, STRIP], _dt.float32,
                                        tag="ps%d" % j, name="ps%d" % j)
                        pss.append(ps)
                        nc.tensor.matmul(
                            out=ps[:, 0:cw],
                            lhsT=wts_t[:, 0:C],
                            rhs=xT_t[:, off + j * STRIP:off + j * STRIP + cw],
                            start=True, stop=(mode == "gather_only"),
                        )
                    if mode == "gather_only":
                        pass
                    else:
                     for c in range(K // 2):
                        for t in range(2):
                            k = 2 * c + t + 1
                            rbase = t * ranks
                            for j in range(nstrip):
                                cw = min(STRIP, nbp - j * STRIP)
                                nch = cw // 128
                                st = spool.tile([C, STRIP], _dt.bfloat16)
                                for u in range(nch):
                                    nc.tensor.transpose(
                                        st[:, u * 128:(u + 1) * 128],
                                        gs[c][:, rbase + j * (STRIP // 128) + u, :],
                                        id_t[:],
                                    )
                                rt = rpool.tile([C, STRIP], _dt.bfloat16)
                                if cctr % 2 == 0:
                                    nc.scalar.activation(
                                        rt[:, 0:cw], st[:, 0:cw],
                                        mybir.ActivationFunctionType.Identity,
                                        scale=1.0,
                                    )
                                else:
                                    nc.vector.tensor_copy(rt[:, 0:cw], st[:, 0:cw])
                                cctr += 1
                                nc.tensor.matmul(
                                    out=pss[j][:, 0:cw],
                                    lhsT=wts_t[:, k * C:(k + 1) * C],
                                    rhs=rt[:, 0:cw],
                                    start=False, stop=(k == K),
                                )
                    for j in range(nstrip):
                        cwo = min(STRIP, nb - j * STRIP)
                        if cwo <= 0:
                            continue
                        o = opool.tile([C, STRIP], _dt.float16)
                        nc.scalar.activation(
                            o[:, 0:cwo], pss[j][:, 0:cwo],
                            mybir.ActivationFunctionType.Identity,
                            bias=bias_t[:], scale=1.0,
                        )
                        nc.sync.dma_start(
                            out=outT[:, off + j * STRIP:off + j * STRIP + cwo],
                            in_=o[:, 0:cwo])
    nc.compile()
    return nc


def fold_weights(W1_w, W1_b, Wc_w, Wc_b, W2_w, W2_b):
    """Collapse Linear->Conv1d->Linear into 11 [C,C] mats + one bias."""
    W2 = W2_w.astype(np.float64)
    M = np.einsum('de,eck->cdk', W2, Wc_w.astype(np.float64))
    M[:, :, 0] += W1_w.T.astype(np.float64)
    wts = np.concatenate([M[:, :, k] for k in range(K + 1)], axis=1)
    bias = W1_b.astype(np.float64) + W2_b.astype(np.float64) + W2 @ Wc_b.astype(np.float64)
    return wts.astype(np.float32), bias.astype(np.float32).reshape(C, 1)


def make_idx(adj_b, n=N):
    """adj[b] [n,K] -> wrapped int16 gather-index stream [128, TOT_SLOTS].

    One dma_gather call per (node-block, k-pair): idx j in [0,nbp) is
    neighbor slot 2c of node off+j, j in [nbp,2*nbp) slot 2c+1 (pad
    positions use index 0; their columns land beyond the block's real
    nodes and are never read back). Index j of a call comes from
    partition j%16, slot j//16, replicated across the 8 GPSIMD groups.
    """
    a = np.asarray(adj_b).astype(np.int16)
    cols = []
    for (off, nb, nbp) in _blocks(n):
        gpc = 2 * nbp
        for c in range(K // 2):
            j = np.zeros(gpc, dtype=np.int16)
            j[:nb] = a[off:off + nb, 2 * c]
            j[nbp:nbp + nb] = a[off:off + nb, 2 * c + 1]
            cols.append(j.reshape(gpc // 16, 16).T)    # [16, slots]
    blk = np.concatenate(cols, axis=1)                 # [16, TOT_SLOTS]
    return np.tile(blk, (8, 1)).copy()                 # replicate 8x


def prep_core_inputs(x, adj_mat, wts, bias):
    """Per-core (per-graph) input maps for the SPMD launch."""
    maps = []
    for b in range(B):
        xb = np.asarray(x[b], dtype=np.float32)
        xb16 = np.ascontiguousarray(xb).astype(BF16)
        xTpad = np.zeros((C, NPAD2), dtype=BF16)
        xTpad[:, :N] = xb.T.astype(BF16)
        maps.append({
            "x_rows": xb16,
            "xT": np.ascontiguousarray(xTpad),
            "idx": make_idx(adj_mat[b]),
            "wts": wts.astype(BF16),
            "bias": bias,
            "ident": np.eye(C, dtype=BF16),
        })
    return maps


_NC_CACHE = {}


def kernel(x, adj_mat, W1_w, W1_b, Wc_w, Wc_b, W2_w, W2_b):
    x = np.asarray(x)
    adj_mat = np.asarray(adj_mat)
    wts, bias = fold_weights(np.asarray(W1_w), np.asarray(W1_b), np.asarray(Wc_w),
                             np.asarray(Wc_b), np.asarray(W2_w), np.asarray(W2_b))
    if "nc" not in _NC_CACHE:
        _NC_CACHE["nc"] = build()
    nc = _NC_CACHE["nc"]
    in_maps = prep_core_inputs(x, adj_mat, wts, bias)
    # The program is bit-deterministic (fixed accumulation order), but a
    # rare transient corruption was observed once in ~25 HW runs (wrong
    # gather data on one call; shared-host DMA glitch). Run until two
    # consecutive executions agree bit-exactly — a transient cannot
    # reproduce identically twice.
    prev = None
    for _attempt in range(5):
        res = run_bass_kernel_spmd(nc, in_maps, list(range(N_CORES)))
        cur = np.stack([np.asarray(res.results[b]["outT"]) for b in range(B)])
        if prev is not None and np.array_equal(cur, prev):
            break
        prev = cur
    out = np.empty((B, N, C), dtype=np.float32)
    for b in range(B):
        out[b] = cur[b].T.astype(np.float32)
    return out
